# revision 2
# baseline (speedup 1.0000x reference)
"""GIN message-passing network (3 GINConv layers + per-graph sum readout) on
8 Trainium2 NeuronCores via Bass/Tile.

Sharding: nodes are partitioned contiguously across the 8 cores (graph
partitioning by destination). Edges are assigned to the core owning their dst
node and sorted per 128-row dst tile. Per layer, each core:
  - aggregates incoming-edge source features with indirect-DMA row gathers
    from a replicated full copy of h, reduced per dst tile via one-hot
    matmuls on the tensor engine (PSUM accumulation),
  - adds the node's own features (identity matmul),
  - applies the dense layer as z @ W + b with PE transposes for the
    stationary operand, fp32r matmuls at full rate,
  - ReLU on the scalar engine.
Between layers the per-core h slabs are AllGathered (rank-concat layout
matches the padded node layout exactly). After layer 3 each core computes
s = h3 @ Wc per node via DVE multiply+reduce; the host does the final
per-graph segment sum (bincount over 50k scalars) and adds bc.

Weights are replicated; all matmul operands are float32r (full-rate fp32
variant, ~1e-4 relative rounding).
"""
import os
import sys
import math

sys.path.insert(0, "/opt/trn_rl_repo")

import numpy as np

import concourse.bass as bass
import concourse.bacc as bacc
import concourse.mybir as mybir
import concourse.tile as tile
from concourse import bass_utils

NC = 8
P = 128

f32 = mybir.dt.float32
f32r = mybir.dt.float32r
i32 = mybir.dt.int32

PAD_DLOC = 200.0  # one-hot no-match sentinel for padding slots

_program_cache = {}
LAST_EXEC_TIME_NS = None
LAST_RESULTS = None


def _build_schedule(src, dst, N, q, T, n_own):
    """Partition edges by dst core, group per 128-row dst tile, chunk by 128.

    Returns (K, O, C, src_idx_T, dloc_T):
      K[t]       chunks for dst-tile t (max over cores, shared static schedule)
      O[t]       chunk offset of tile t
      C          total chunks
      src_idx_T  [NC, P, C] int32 padded-global source ids (pad: 0)
      dloc_T     [NC, P, C] f32 dst index within tile (pad: PAD_DLOC)
    """
    E = src.shape[0]
    c_e = dst // q
    dl = dst - c_e * q
    t_e = dl // P
    m_e = dl - t_e * P
    sc = src // q
    srcp = sc * n_own + (src - sc * q)

    key = c_e * T + t_e
    order = np.argsort(key, kind="stable")
    key_s = key[order]
    srcp_s = srcp[order]
    m_s = m_e[order]

    counts = np.bincount(key, minlength=NC * T)
    starts = np.concatenate([[0], np.cumsum(counts)])
    rank = np.arange(E, dtype=np.int64) - starts[key_s]

    K = np.ceil(counts.reshape(NC, T).max(axis=0) / P).astype(np.int64)
    O = np.concatenate([[0], np.cumsum(K)])
    C = int(O[-1])

    j = rank // P
    s = rank - j * P
    c_s = key_s // T
    t_s = key_s - c_s * T
    col = O[t_s] + j

    src_idx_T = np.zeros((NC, P, C), np.int32)
    dloc_T = np.full((NC, P, C), PAD_DLOC, np.float32)
    src_idx_T[c_s, s, col] = srcp_s.astype(np.int32)
    dloc_T[c_s, s, col] = m_s.astype(np.float32)
    return K, O, C, src_idx_T, dloc_T


def _build_program(D, T, C, K, O, n_own, N_pad):
    KT = D // P
    nc = bacc.Bacc("TRN2", target_bir_lowering=False, debug=False,
                   num_devices=NC)

    x_full = nc.dram_tensor("x_full", [N_pad, D], f32r, kind="ExternalInput").ap()
    x_own = nc.dram_tensor("x_own", [n_own, D], f32r, kind="ExternalInput").ap()
    w_in = [nc.dram_tensor(f"w{l}_in", [D, D], f32r, kind="ExternalInput").ap()
            for l in range(3)]
    b_in = [nc.dram_tensor(f"b{l}_in", [1, D], f32r, kind="ExternalInput").ap()
            for l in range(3)]
    wc_in = nc.dram_tensor("wc_in", [P, D], f32, kind="ExternalInput").ap()
    colidx_in = nc.dram_tensor("colidx_in", [P, P], f32, kind="ExternalInput").ap()
    ident_in = nc.dram_tensor("ident_in", [P, P], f32r, kind="ExternalInput").ap()
    ones_in = nc.dram_tensor("ones_in", [1, P], f32r, kind="ExternalInput").ap()
    idx_in = nc.dram_tensor("idx_in", [P, C], i32, kind="ExternalInput").ap()
    dloc_in = nc.dram_tensor("dloc_in", [P, C], f32, kind="ExternalInput").ap()
    s_out = nc.dram_tensor("s_out", [n_own, 1], f32, kind="ExternalOutput").ap()

    with tile.TileContext(nc) as tc:
        with tc.tile_pool(name="const", bufs=1) as const, \
             tc.tile_pool(name="dram", bufs=1, space="DRAM") as dram, \
             tc.tile_pool(name="gpool", bufs=8) as gpool, \
             tc.tile_pool(name="opool", bufs=8) as opool, \
             tc.tile_pool(name="work", bufs=3) as work, \
             tc.tile_pool(name="zpsum", bufs=2, space="PSUM") as zpsum, \
             tc.tile_pool(name="tpsum", bufs=2, space="PSUM") as tpsum, \
             tc.tile_pool(name="ypsum", bufs=2, space="PSUM") as ypsum:

            # ------- resident constants
            colidx_sb = const.tile([P, P], f32)
            nc.sync.dma_start(out=colidx_sb[:], in_=colidx_in[:])
            ident_sb = const.tile([P, P], f32r)
            nc.sync.dma_start(out=ident_sb[:], in_=ident_in[:])
            ones_sb = const.tile([1, P], f32r)
            nc.sync.dma_start(out=ones_sb[:], in_=ones_in[:])
            wc_sb = const.tile([P, D], f32)
            nc.sync.dma_start(out=wc_sb[:], in_=wc_in[:])
            idx_sb = const.tile([P, C], i32)
            nc.sync.dma_start(out=idx_sb[:], in_=idx_in[:])
            dloc_sb = const.tile([P, C], f32)
            nc.sync.dma_start(out=dloc_sb[:], in_=dloc_in[:])
            w_sb = []
            b_sb = []
            for l in range(3):
                w_l = const.tile([P, KT * D], f32r, name=f"w_sb{l}")
                for k in range(KT):
                    nc.sync.dma_start(out=w_l[:, k * D:(k + 1) * D],
                                      in_=w_in[l][k * P:(k + 1) * P, :])
                w_sb.append(w_l)
                b_l = const.tile([1, D], f32r, name=f"b_sb{l}")
                nc.sync.dma_start(out=b_l[:], in_=b_in[l][:])
                b_sb.append(b_l)

            # ------- inter-layer DRAM
            h_own_a = dram.tile([n_own, D], f32r)
            h_own_b = dram.tile([n_own, D], f32r)
            h_full_a = dram.tile([N_pad, D], f32r, addr_space="Shared")
            h_full_b = dram.tile([N_pad, D], f32r, addr_space="Shared")

            def emit_layer(l, h_full_ap, h_own_ap, out_own_ap):
                for t in range(T):
                    nch = int(K[t])
                    psum_z = zpsum.tile([P, D], f32, space="PSUM", name="psum_z")
                    for j in range(nch):
                        col = int(O[t]) + j
                        g = gpool.tile([P, D], f32r, name="g")
                        nc.gpsimd.indirect_dma_start(
                            out=g[:], out_offset=None, in_=h_full_ap[:],
                            in_offset=bass.IndirectOffsetOnAxis(
                                ap=idx_sb[:, col:col + 1], axis=0),
                        )
                        oh = opool.tile([P, P], f32r, name="oh")
                        nc.vector.tensor_tensor(
                            out=oh[:],
                            in0=dloc_sb[:, col:col + 1].to_broadcast([P, P]),
                            in1=colidx_sb[:], op=mybir.AluOpType.is_equal)
                        nc.tensor.matmul(out=psum_z[:], lhsT=oh[:], rhs=g[:],
                                         start=(j == 0), stop=False)
                    h_own_t = work.tile([P, D], f32r, name="h_own_t")
                    nc.sync.dma_start(out=h_own_t[:],
                                      in_=h_own_ap[t * P:(t + 1) * P, :])
                    nc.tensor.matmul(out=psum_z[:], lhsT=ident_sb[:],
                                     rhs=h_own_t[:], start=(nch == 0), stop=True)

                    z_sb = work.tile([P, D], f32r, name="z_sb")
                    nc.vector.tensor_copy(out=z_sb[:], in_=psum_z[:])
                    zt_sb = work.tile([P, D], f32r, name="zt_sb")
                    for k in range(KT):
                        zt_ps = tpsum.tile([P, P], f32r, space="PSUM",
                                           name="zt_ps")
                        nc.tensor.transpose(out=zt_ps[:],
                                            in_=z_sb[:, k * P:(k + 1) * P],
                                            identity=ident_sb[:])
                        nc.vector.tensor_copy(out=zt_sb[:, k * P:(k + 1) * P],
                                              in_=zt_ps[:])

                    psum_y = ypsum.tile([P, D], f32, space="PSUM", name="psum_y")
                    for k in range(KT):
                        nc.tensor.matmul(out=psum_y[:],
                                         lhsT=zt_sb[:, k * P:(k + 1) * P],
                                         rhs=w_sb[l][:, k * D:(k + 1) * D],
                                         start=(k == 0), stop=False)
                    nc.tensor.matmul(out=psum_y[:], lhsT=ones_sb[:],
                                     rhs=b_sb[l][:], start=False, stop=True)

                    h_sb = work.tile([P, D], f32, name="h_sb")
                    nc.scalar.activation(out=h_sb[:], in_=psum_y[:],
                                         func=mybir.ActivationFunctionType.Relu)
                    if out_own_ap is not None:
                        nc.sync.dma_start(
                            out=out_own_ap[t * P:(t + 1) * P, :],
                            in_=h_sb[:].bitcast(f32r))
                    else:
                        scratch = work.tile([P, D], f32, name="scratch")
                        nc.vector.tensor_tensor(out=scratch[:], in0=h_sb[:],
                                                in1=wc_sb[:],
                                                op=mybir.AluOpType.mult)
                        s_sb = work.tile([P, 1], f32, name="s_sb")
                        nc.vector.reduce_sum(out=s_sb[:], in_=scratch[:],
                                             axis=mybir.AxisListType.X)
                        nc.sync.dma_start(out=s_out[t * P:(t + 1) * P, :],
                                          in_=s_sb[:])

            def emit_ag(h_own_t_, h_full_t_):
                nc.gpsimd.collective_compute(
                    "AllGather", mybir.AluOpType.bypass,
                    replica_groups=[list(range(NC))],
                    ins=[h_own_t_.opt()], outs=[h_full_t_.opt()],
                )

            emit_layer(0, x_full, x_own, h_own_a[:])
            emit_ag(h_own_a, h_full_a)
            emit_layer(1, h_full_a[:], h_own_a[:], h_own_b[:])
            emit_ag(h_own_b, h_full_b)
            emit_layer(2, h_full_b[:], h_own_b[:], None)

    nc.compile()
    return nc


def kernel(node_features, src, dst, graph_ids, num_graphs,
           W1, b1, W2, b2, W3, b3, Wc, bc):
    global LAST_EXEC_TIME_NS, LAST_RESULTS

    x = np.ascontiguousarray(np.asarray(node_features, dtype=np.float32))
    src = np.asarray(src).astype(np.int64)
    dst = np.asarray(dst).astype(np.int64)
    gids = np.asarray(graph_ids).astype(np.int64)
    G = int(np.asarray(num_graphs))
    W = [np.ascontiguousarray(np.asarray(w, np.float32)) for w in (W1, W2, W3)]
    b = [np.asarray(x_, np.float32).reshape(1, -1) for x_ in (b1, b2, b3)]
    wc = np.asarray(Wc, np.float32).reshape(-1)
    bc_v = np.asarray(bc, np.float32).reshape(-1)[0]

    N, D = x.shape
    q = math.ceil(N / NC)
    T = math.ceil(q / P)
    n_own = T * P
    N_pad = NC * n_own

    K, O, C, src_idx_T, dloc_T = _build_schedule(src, dst, N, q, T, n_own)

    sig = (N, D, C, n_own, tuple(int(k) for k in K))
    if sig not in _program_cache:
        _program_cache[sig] = _build_program(D, T, C, K, O, n_own, N_pad)
    nc = _program_cache[sig]

    # padded per-core slabs; rank-concat of slabs == padded full layout
    x_own = np.zeros((NC, n_own, D), np.float32)
    for c in range(NC):
        lo, hi = c * q, min((c + 1) * q, N)
        x_own[c, :hi - lo] = x[lo:hi]
    x_full = np.ascontiguousarray(x_own.reshape(NC * n_own, D))

    wc_rep = np.ascontiguousarray(np.tile(wc[None, :], (P, 1)).astype(np.float32))
    colidx = np.ascontiguousarray(np.tile(np.arange(P, dtype=np.float32), (P, 1)))
    ident = np.eye(P, dtype=np.float32)
    ones = np.ones((1, P), np.float32)

    in_maps = []
    for c in range(NC):
        in_maps.append({
            "x_full": x_full,
            "x_own": np.ascontiguousarray(x_own[c]),
            "w0_in": W[0], "w1_in": W[1], "w2_in": W[2],
            "b0_in": b[0], "b1_in": b[1], "b2_in": b[2],
            "wc_in": wc_rep,
            "colidx_in": colidx,
            "ident_in": ident,
            "ones_in": ones,
            "idx_in": np.ascontiguousarray(src_idx_T[c]),
            "dloc_in": np.ascontiguousarray(dloc_T[c]),
        })

    r = bass_utils.run_bass_kernel_spmd(nc, in_maps,
                                        core_ids=list(range(NC)))
    LAST_EXEC_TIME_NS = r.exec_time_ns
    LAST_RESULTS = r

    parts = []
    for c in range(NC):
        lo, hi = c * q, min((c + 1) * q, N)
        parts.append(r.results[c]["s_out"][:hi - lo, 0])
    s = np.concatenate(parts)
    y = np.bincount(gids, weights=s.astype(np.float64), minlength=G)[:G]
    return (y.astype(np.float32) + bc_v)[:, None]
